# revision 1
# baseline (speedup 1.0000x reference)
"""EvolveGNN-O Trainium2 kernel (8 NeuronCores, SPMD).

Strategy (node-sharded by destination):
- Destination nodes sharded across 8 cores (12500 each); each core owns all
  edges whose col (destination) is in its range.
- Per core: edges sorted by col into 98 windows of 128 destination nodes;
  within a window split into 2 source-row chunks so dma_gather's int16
  indices cover 100k rows via offset table bases (idx = row - base, using the
  full signed int16 range).
- GRU + weight generation computed redundantly on every core (tiny).
- deg histogram: onehot(col) bf16 matmuls against ones (exact for counts).
- y2 = dinv * (x @ W^T) built per shard, AllGather'd into a full 100k-row
  DRAM table.
- Aggregation: dma_gather y2[row] messages (one call per 4-window super-group
  per chunk), onehot(col) f32 built on DVE via broadcast is_equal, accumulated
  in PSUM by TensorE matmuls.
- out = dinv * (agg + y2_local) + bias; host concatenates shards.
"""

import numpy as np
import ml_dtypes

import concourse.bass as bass
import concourse.bacc as bacc
import concourse.mybir as mybir
import concourse.tile as tile
from concourse.bass_utils import run_bass_kernel_spmd
from concourse.masks import make_identity

dt = mybir.dt

N_NODES = 100000
N_EDGES = 1600000
CH = 64
NCORES = 8
NLOC = N_NODES // NCORES          # 12500 dst nodes per core
WSZ = 128
W = (NLOC + WSZ - 1) // WSZ       # 98 windows (last partial: 84)
NPAD = W * WSZ                    # 12544
LAST_W = NLOC - (W - 1) * WSZ     # 84
NCH = 2
CHUNK_LO = (0, 50000)             # chunk row ranges [lo, hi)
CHUNK_HI = (50000, 100000)
CHUNK_BASE = (32768, 82768)       # table base row; idx = row - base (int16)
SUPER = 4                         # windows per gather super-group

_BUILD_CACHE: dict = {}


def _supers():
    out = []
    w = 0
    while w < W:
        out.append((w, min(w + SUPER, W)))
        w += SUPER
    return out


def _build(Ks: tuple) -> "bacc.Bacc":
    """Ks[g]: slots per (window, chunk g), multiple of 128."""
    S = sum(Ks)                   # slots per window
    TPW = S // 128                # tiles per window (= sum of per-chunk tiles)
    TG = tuple(k // 128 for k in Ks)
    TT = W * TPW
    IDXC = (W * sum(Ks) + 256) // 16
    supers = _supers()

    nc = bacc.Bacc("TRN2", target_bir_lowering=False, debug=False,
                   num_devices=NCORES)

    # ---- inputs ----
    x_sh = nc.dram_tensor("x_sh", [128, W * CH], dt.float32, kind="ExternalInput")
    colrel = nc.dram_tensor("colrel", [128, TT], dt.bfloat16, kind="ExternalInput")
    idx_in = nc.dram_tensor("idx_in", [128, IDXC], dt.int16, kind="ExternalInput")
    mw_in = nc.dram_tensor("mw_in", [64], dt.float32, kind="ExternalInput")
    wih_in = nc.dram_tensor("wih_in", [128, 2 * CH], dt.float32, kind="ExternalInput")
    bih_in = nc.dram_tensor("bih_in", [192], dt.float32, kind="ExternalInput")
    bhh_in = nc.dram_tensor("bhh_in", [192], dt.float32, kind="ExternalInput")
    wtw_in = nc.dram_tensor("wtw_in", [128, 32 * CH], dt.float32, kind="ExternalInput")
    wtb_in = nc.dram_tensor("wtb_in", [4096], dt.float32, kind="ExternalInput")
    gbias_in = nc.dram_tensor("gbias_in", [64], dt.float32, kind="ExternalInput")

    out_d = nc.dram_tensor("out_d", [128, W * CH], dt.float32, kind="ExternalOutput")
    dbg = bool(int(__import__("os").environ.get("GNN_DEBUG", "0")))
    if dbg:
        deg_o = nc.dram_tensor("deg_o", [128, W], dt.float32, kind="ExternalOutput")
        y2f_o = nc.dram_tensor("y2f_o", [N_NODES, CH], dt.float32, kind="ExternalOutput")

    y2_shard = nc.dram_tensor("y2_shard", [NLOC, CH], dt.float32)
    y2_full = nc.dram_tensor("y2_full", [N_NODES, CH], dt.float32, addr_space="Shared")

    with tile.TileContext(nc) as tc:
        with (
            tc.tile_pool(name="res", bufs=1) as res,
            tc.tile_pool(name="work", bufs=2) as work,
            tc.tile_pool(name="msgsp", bufs=6) as msgsp,
        ):
            # ---- resident ----
            col_sb = res.tile([128, TT], dt.bfloat16)
            nc.sync.dma_start(col_sb[:], colrel[:])
            idx_sb = res.tile([128, IDXC], dt.int16)
            nc.sync.dma_start(idx_sb[:], idx_in[:])
            bias_sb = res.tile([128, CH], dt.float32)
            nc.sync.dma_start(bias_sb[:], gbias_in[None, :].to_broadcast([128, CH]))
            iota_b = res.tile([128, 128], dt.bfloat16)
            nc.gpsimd.iota(iota_b[:], pattern=[[1, 128]], base=0,
                           channel_multiplier=0, allow_small_or_imprecise_dtypes=True)
            ident = res.tile([128, 128], dt.float32)
            make_identity(nc, ident[:])
            ones_b = res.tile([128, 1], dt.bfloat16)
            nc.vector.memset(ones_b[:], 1.0)

            y2_sb = res.tile([128, W, CH], dt.float32)
            deg_sb = res.tile([128, W], dt.float32)
            dinv_sb = res.tile([128, W], dt.float32)
            WT_sb = res.tile([64, 64], dt.float32)

            # ---- phase A: W generation ----
            with tc.tile_pool(name="psA", bufs=2, space="PSUM") as psA:
                wih_sb = work.tile([128, 2, CH], dt.float32, tag="wih")
                nc.sync.dma_start(wih_sb[:], wih_in[:].rearrange("p (t c) -> p t c", c=CH))
                wihT_sb = work.tile([64, 256], dt.float32, tag="wihT")
                for t in range(2):
                    trp = psA.tile([64, 128], dt.float32, space="PSUM", tag="tr")
                    nc.tensor.transpose(trp[:], wih_sb[:, t, :], ident[:])
                    nc.vector.tensor_copy(wihT_sb[:, 128 * t:128 * (t + 1)], trp[:])

                mw_sb = work.tile([64, 1], dt.float32, tag="mw")
                nc.sync.dma_start(mw_sb[:], mw_in[:, None])
                bih_sb = work.tile([64, 3], dt.float32, tag="bih")
                nc.sync.dma_start(bih_sb[:], bih_in[:].rearrange("(s p) -> p s", p=64))
                bhh_sb = work.tile([64, 3], dt.float32, tag="bhh")
                nc.sync.dma_start(bhh_sb[:], bhh_in[:].rearrange("(s p) -> p s", p=64))

                gi_sb = work.tile([64, 3], dt.float32, tag="gi")
                for s in range(3):
                    gps = psA.tile([64, 1], dt.float32, space="PSUM", tag="gi")
                    nc.tensor.matmul(gps[:], wihT_sb[:, 64 * s:64 * (s + 1)],
                                     mw_sb[:], start=True, stop=True)
                    nc.vector.tensor_copy(gi_sb[:, s:s + 1], gps[:])

                bsum = work.tile([64, 2], dt.float32, tag="bsum")
                nc.vector.tensor_add(bsum[:], bih_sb[:, 0:2], bhh_sb[:, 0:2])
                gates = work.tile([64, 4], dt.float32, tag="gates")
                nc.scalar.activation(gates[:, 0:1], gi_sb[:, 0:1],
                                     mybir.ActivationFunctionType.Sigmoid,
                                     bias=bsum[:, 0:1])
                nc.scalar.activation(gates[:, 1:2], gi_sb[:, 1:2],
                                     mybir.ActivationFunctionType.Sigmoid,
                                     bias=bsum[:, 1:2])
                nb = work.tile([64, 1], dt.float32, tag="nb")
                nc.vector.tensor_mul(nb[:], gates[:, 0:1], bhh_sb[:, 2:3])
                nc.vector.tensor_add(nb[:], nb[:], bih_sb[:, 2:3])
                nc.scalar.activation(gates[:, 2:3], gi_sb[:, 2:3],
                                     mybir.ActivationFunctionType.Tanh, bias=nb[:])
                omz = work.tile([64, 1], dt.float32, tag="omz")
                nc.vector.tensor_scalar(omz[:], gates[:, 1:2], -1.0, 1.0,
                                        mybir.AluOpType.mult, mybir.AluOpType.add)
                um_sb = work.tile([64, 1], dt.float32, tag="um")
                nc.vector.tensor_mul(um_sb[:], omz[:], gates[:, 2:3])

                wtw_sb = work.tile([128, 32, CH], dt.float32, tag="wtw")
                nc.sync.dma_start(wtw_sb[:], wtw_in[:].rearrange("p (t c) -> p t c", c=CH))
                wtbT_sb = work.tile([64, 64], dt.float32, tag="wtbT")
                nc.sync.dma_start(wtbT_sb[:], wtb_in[:].rearrange("(o p) -> p o", p=64))
                W_ps = psA.tile([64, 64], dt.float32, space="PSUM", tag="W")
                for t in range(32):
                    trp = psA.tile([64, 128], dt.float32, space="PSUM", tag="tr")
                    nc.tensor.transpose(trp[:], wtw_sb[:, t, :], ident[:])
                    trs = work.tile([64, 128], dt.float32, tag="trs")
                    nc.vector.tensor_copy(trs[:], trp[:])
                    for b in range(2):
                        nc.tensor.matmul(W_ps[:, 2 * t + b:2 * t + b + 1],
                                         trs[:, 64 * b:64 * (b + 1)], um_sb[:],
                                         start=True, stop=True,
                                         skip_group_check=True)
                nc.vector.tensor_add(WT_sb[:], W_ps[:], wtbT_sb[:])

                # ---- phase A2: deg (bf16 onehots, one build per super) ----
                for (w0, w1) in supers:
                    ns = w1 - w0
                    ntile = ns * TPW
                    oh = work.tile([128, SUPER * TPW, 128], dt.bfloat16, tag="degoh")
                    tbase = w0 * TPW
                    nc.vector.tensor_tensor(
                        out=oh[:, :ntile, :],
                        in0=col_sb[:, tbase:tbase + ntile].unsqueeze(2)
                            .to_broadcast([128, ntile, 128]),
                        in1=iota_b[:].unsqueeze(1).to_broadcast([128, ntile, 128]),
                        op=mybir.AluOpType.is_equal)
                    for wi in range(ns):
                        dps = psA.tile([128, 1], dt.float32, space="PSUM", tag="deg")
                        # col-tile order is window-major: window wi's tiles are
                        # [wi*TPW, (wi+1)*TPW) = (chunk0 j.., chunk1 j..)
                        for k in range(TPW):
                            nc.tensor.matmul(dps[:], oh[:, wi * TPW + k, :],
                                             ones_b[:], start=(k == 0),
                                             stop=(k == TPW - 1))
                        nc.vector.tensor_copy(deg_sb[:, w0 + wi:w0 + wi + 1], dps[:])

                if dbg:
                    nc.sync.dma_start(deg_o[:], deg_sb[:])
                sq = work.tile([128, W], dt.float32, tag="sq")
                nc.scalar.activation(sq[:], deg_sb[:],
                                     mybir.ActivationFunctionType.Sqrt, bias=1.0)
                nc.vector.reciprocal(dinv_sb[:], sq[:])

            # ---- phase B: y2 = dinv * (x @ W^T) ----
            with tc.tile_pool(name="psB", bufs=2, space="PSUM") as psB:
                XG = 8
                for w0 in range(0, W, XG):
                    w1 = min(w0 + XG, W)
                    xg = work.tile([128, XG, CH], dt.float32, tag="xg")
                    nc.sync.dma_start(
                        xg[:, :w1 - w0, :],
                        x_sh[:, w0 * CH:w1 * CH].rearrange("p (w c) -> p w c", c=CH))
                    for w in range(w0, w1):
                        xTp = psB.tile([64, 128], dt.float32, space="PSUM", tag="xT")
                        nc.tensor.transpose(xTp[:], xg[:, w - w0, :], ident[:])
                        xTs = work.tile([64, 128], dt.float32, tag="xTs")
                        nc.vector.tensor_copy(xTs[:], xTp[:])
                        xwp = psB.tile([128, CH], dt.float32, space="PSUM", tag="xw")
                        nc.tensor.matmul(xwp[:], xTs[:], WT_sb[:], start=True, stop=True)
                        nc.vector.tensor_scalar_mul(y2_sb[:, w, :], xwp[:],
                                                    dinv_sb[:, w:w + 1])

            nc.sync.dma_start(
                y2_shard[0:(W - 1) * WSZ, :].rearrange("(w p) c -> p w c", p=128),
                y2_sb[:, 0:W - 1, :])
            nc.sync.dma_start(y2_shard[(W - 1) * WSZ:NLOC, :],
                              y2_sb[0:LAST_W, W - 1, :])
            nc.gpsimd.collective_compute(
                "AllGather", mybir.AluOpType.bypass,
                replica_groups=[list(range(NCORES))],
                ins=[y2_shard[:]], outs=[y2_full[:]])
            if dbg:
                nc.sync.dma_start(y2f_o[:], y2_full[:])

            # ---- phase C: gather + aggregate ----
            # Slot layout is chunk-major: chunk g's slots are the W windows'
            # K_g-runs back to back. Gather calls cover 7 tiles (896 tokens)
            # but only the first 6 are consumed: under sustained SWDGE
            # pressure the tail tokens of a call (top partitions of its last
            # tile) intermittently corrupt, so each call's 7th tile is a
            # redundant prefetch of the next call's first tile and is
            # discarded. Each chunk's token stream carries a 128-token
            # sacrificial tail for the final call. (1024-token calls also hit
            # a ring-edge bug; idx slice offsets must stay 32B-aligned ->
            # the 48-column stride is fine.)
            CT = 6
            with tc.tile_pool(name="psC", bufs=2, space="PSUM") as psC:
                call_tiles = [[] for _ in range(NCH)]   # per chunk: tile objs
                emitted = [0, 0]                        # consumed-tiles emitted
                idx_base = [0, (W * Ks[0] + 128) // 16]  # idx col offsets
                total_tiles = [W * TG[g] for g in range(NCH)]

                def emit_gathers(g, need_tiles):
                    while emitted[g] < min(need_tiles, total_tiles[g]):
                        c = len(call_tiles[g])
                        ncons = min(CT, total_tiles[g] - emitted[g])
                        nk = (ncons + 1) * 128
                        mt = msgsp.tile([128, CT + 1, CH], dt.float32,
                                        tag=f"msgs{g}")
                        c0 = idx_base[g] + c * 48
                        nc.gpsimd.dma_gather(
                            mt[:, :ncons + 1, :], y2_full[CHUNK_BASE[g]:, :],
                            idx_sb[:, c0:c0 + nk // 16], nk, nk, CH)
                        call_tiles[g].append(mt)
                        emitted[g] += ncons

                for w in range(W):
                    for g in range(NCH):
                        emit_gathers(g, (w + 1) * TG[g])
                    oh = work.tile([128, TPW, 128], dt.float32, tag="aggoh")
                    tbase = w * TPW
                    nc.vector.tensor_tensor(
                        out=oh[:],
                        in0=col_sb[:, tbase:tbase + TPW].unsqueeze(2)
                            .to_broadcast([128, TPW, 128]),
                        in1=iota_b[:].unsqueeze(1).to_broadcast([128, TPW, 128]),
                        op=mybir.AluOpType.is_equal)
                    aps = psC.tile([128, CH], dt.float32, space="PSUM", tag="agg")
                    k = 0
                    for g in range(NCH):
                        for j in range(TG[g]):
                            gt = w * TG[g] + j
                            mt = call_tiles[g][gt // CT]
                            nc.tensor.matmul(aps[:], oh[:, (0 if g == 0 else TG[0]) + j, :],
                                             mt[:, gt % CT, :],
                                             start=(k == 0),
                                             stop=(k == TPW - 1))
                            k += 1
                    ot = work.tile([128, CH], dt.float32, tag="ot")
                    nc.vector.tensor_add(ot[:], aps[:], y2_sb[:, w, :])
                    nc.vector.tensor_scalar_mul(ot[:], ot[:], dinv_sb[:, w:w + 1])
                    nc.vector.tensor_add(ot[:], ot[:], bias_sb[:])
                    nc.sync.dma_start(out_d[:, w * CH:(w + 1) * CH], ot[:])

    nc.compile()
    return nc


def _host_prep(x, edge_index, memory_weights, gru_w_ih, gru_b_ih, gru_b_hh,
               wt_w, wt_b, gcn_bias):
    rows = np.asarray(edge_index[0], dtype=np.int64)
    cols = np.asarray(edge_index[1], dtype=np.int64)
    x = np.asarray(x, dtype=np.float32)

    order = np.argsort(cols, kind="stable")
    rows_s = rows[order].astype(np.int32)
    cols_s = cols[order].astype(np.int32)
    core_bounds = np.searchsorted(cols_s, np.arange(NCORES + 1) * NLOC)

    per_core = []
    cnt_stack = []
    for j in range(NCORES):
        lo, hi = core_bounds[j], core_bounds[j + 1]
        ec = cols_s[lo:hi] - j * NLOC
        er = rows_s[lo:hi]
        w = ec >> 7
        g = (er >= CHUNK_LO[1]).astype(np.int64)
        cell = w * NCH + g
        cnt = np.bincount(cell, minlength=W * NCH).astype(np.int64)
        per_core.append((ec, er, cell, cnt))
        cnt_stack.append(cnt)
    cnt_max = np.max(np.stack(cnt_stack), axis=0).reshape(W, NCH)
    Ks = tuple(int(np.ceil(cnt_max[:, g].max() / 128) * 128) for g in range(NCH))
    S = sum(Ks)
    TPW = S // 128
    TG = tuple(k // 128 for k in Ks)
    supers = _supers()

    # chunk-major slot layout: chunk 0 block = W windows' K0-runs back to
    # back, then chunk 1 block. cell (w, g) base = g_block + w*Ks[g].
    g_block = np.array([0, W * Ks[0]], np.int64)
    base_off = np.zeros(W * NCH, np.int64)
    for w in range(W):
        for g in range(NCH):
            base_off[w * NCH + g] = g_block[g] + w * Ks[g]
    TOTSLOT = W * S

    in_maps = []
    for j in range(NCORES):
        ec, er, cell, cnt = per_core[j]
        colrel = np.full(TOTSLOT, -1.0, np.float32)
        idxs = np.zeros(TOTSLOT, np.int16)   # pad idx 0 -> valid row, onehot 0
        cello = np.argsort(cell, kind="stable")
        cs = np.zeros(W * NCH + 1, np.int64)
        np.cumsum(cnt, out=cs[1:])
        ranks = np.empty(len(cell), np.int64)
        ranks[cello] = np.arange(len(cell)) - cs[cell[cello]]
        slot = base_off[cell] + ranks
        colrel[slot] = (ec & 127).astype(np.float32)
        idxs[slot] = (er - np.where(er >= CHUNK_LO[1], CHUNK_BASE[1],
                                    CHUNK_BASE[0])).astype(np.int16)

        # idx stream per chunk = its tokens + a 128-token sacrificial tail
        # (idx 0) covering the final call's discarded 7th tile.
        parts = []
        for g in range(NCH):
            parts.append(idxs[g_block[g]:g_block[g] + W * Ks[g]])
            parts.append(np.zeros(128, np.int16))
        idx_stream = np.concatenate(parts)
        idx_cols = idx_stream.reshape(len(idx_stream) // 16, 16).T
        idx_rep = np.tile(idx_cols, (8, 1)).copy()

        # col tiles in window-major order: window w's tiles are
        # [w*TPW, (w+1)*TPW) ordered (chunk0 j.., chunk1 j..)
        col_tiles = np.empty((128, W * TPW), np.float32)
        for g in range(NCH):
            cr = colrel[g_block[g]:g_block[g] + W * Ks[g]] \
                .reshape(W, TG[g], 128)
            tb = 0 if g == 0 else TG[0]
            for w in range(W):
                col_tiles[:, w * TPW + tb:w * TPW + tb + TG[g]] = cr[w].T

        xp = np.zeros((NPAD, CH), np.float32)
        xp[:NLOC] = x[j * NLOC:(j + 1) * NLOC]
        x_shuf = xp.reshape(W, 128, CH).transpose(1, 0, 2).reshape(128, W * CH).copy()

        wih_p = np.zeros((256, CH), np.float32)
        wih_p[:192] = np.asarray(gru_w_ih, np.float32)
        wih_shuf = wih_p.reshape(2, 128, CH).transpose(1, 0, 2).reshape(128, 2 * CH).copy()
        wtw = np.asarray(wt_w, np.float32)
        wtw_shuf = wtw.reshape(32, 128, CH).transpose(1, 0, 2).reshape(128, 32 * CH).copy()

        in_maps.append(dict(
            x_sh=x_shuf,
            colrel=col_tiles.astype(ml_dtypes.bfloat16),
            idx_in=idx_rep,
            mw_in=np.asarray(memory_weights, np.float32),
            wih_in=wih_shuf,
            bih_in=np.asarray(gru_b_ih, np.float32),
            bhh_in=np.asarray(gru_b_hh, np.float32),
            wtw_in=wtw_shuf,
            wtb_in=np.asarray(wt_b, np.float32),
            gbias_in=np.asarray(gcn_bias, np.float32),
        ))
    return Ks, in_maps


def kernel(x, edge_index, memory_weights, gru_w_ih, gru_w_hh, gru_b_ih,
           gru_b_hh, wt_w, wt_b, gcn_bias, _want_trace=False):
    Ks, in_maps = _host_prep(x, edge_index, memory_weights, gru_w_ih,
                             gru_b_ih, gru_b_hh, wt_w, wt_b, gcn_bias)
    if Ks not in _BUILD_CACHE:
        _BUILD_CACHE[Ks] = _build(Ks)
    nc = _BUILD_CACHE[Ks]
    res = run_bass_kernel_spmd(nc, in_maps, list(range(NCORES)),
                               trace=_want_trace)
    out = np.empty((N_NODES, CH), np.float32)
    for j in range(NCORES):
        o = res.results[j]["out_d"].reshape(128, W, CH).transpose(1, 0, 2)
        out[j * NLOC:(j + 1) * NLOC] = o.reshape(NPAD, CH)[:NLOC]
    kernel._last_result = res
    return out



# revision 39
# speedup vs baseline: 1.2506x; 1.2506x over previous
"""EvolveGNN-O Trainium2 kernel (8 NeuronCores, SPMD): source-sharded.

Strategy (edge-parallel, sharded by source row; the hint's "all-reduce the
per-node segment sums" shape, realized as segmented ReduceScatters):
- out = dinv_c * ((sum_e xd_r + xd_c) @ W^T) + b, where xd = dinv * x. The
  x-message aggregation is W-independent, so GRU/weight-gen overlaps it and
  the generated W applies post-reduction on 12.5k rows/core only.
- Core c owns rows [c*12500, (c+1)*12500): computes xd for them (dinv is
  host-precomputed from edge_index alone), writes a 3.2MB p-major gather
  table (contiguous per-partition write); gathers start ~35us in.
- Its edges sorted by destination into 784 windows of 128 padded-dst
  (8 chunks x 98 local windows). Window slot ranges are NOT tile-aligned:
  adjacent windows share boundary tiles (one onehot build per
  (window, tile) overlap), so slot padding is only the per-window max over
  cores (~8%), not ceil-128 (~50%).
- Aggregation: dma_gather xd[row] messages (896-token calls, 6 of 7 tiles
  consumed — the sacrificial-tail SWDGE-corruption workaround; >1024-token
  calls crash the stack regardless of ring size); onehot(col) f32 built on
  DVE via broadcast is_equal; 7 windows accumulate per PSUM bank; banks
  flushed bf16 on the Act engine into a [seg, chunk, p, lw, ch] DRAM layout
  whose writes are 1KB-contiguous per partition.
- 7 segmented ReduceScatter(add) collectives, one per 14-local-window slab,
  each issued right after its slab's flushes land; a ~10-call gather
  lookahead (deep msg pool) rides out the ~20us each collective blocks the
  Pool queue. Per-segment tails (S = agg + xd; out = dinv*(S@W^T) + bias,
  batched PSUM + broadcast scale/bias) are software-pipelined one segment
  behind, so only the last RS + tail are exposed (~40us).
"""

import numpy as np
import ml_dtypes

import concourse.bass as bass
import concourse.bacc as bacc
import concourse.mybir as mybir
import concourse.tile as tile
from concourse.bass_utils import run_bass_kernel_spmd
from concourse.masks import make_identity

dt = mybir.dt

import os

N_NODES = 100000
N_EDGES = 1600000
CH = 64
NCORES = 8
NLOC = N_NODES // NCORES          # 12500 source rows per core
WL = (NLOC + 127) // 128          # 98 local windows (x/out packing)
NPAD_L = WL * 128                 # 12544
LAST_WL = NLOC - (WL - 1) * 128   # 84
CT = int(os.environ.get("GNN_CT", "6"))    # consumed tiles per gather call
SCRATCH = int(os.environ.get("GNN_SCRATCH", "16384"))
NO_RS = bool(int(os.environ.get("GNN_NO_RS", "0")))
WG = NCORES * WL                  # 784 dst windows over 8 padded 12544 chunks
FB = 7                            # windows per PSUM bank / flush batch
SEG = 7                           # ReduceScatter segments
LWS = WL // SEG                   # local windows per segment (14 = 2*FB)
QL = NCORES * LWS                 # positions per segment (112)

_BUILD_CACHE: dict = {}


def _structure(Ks):
    """Fixed program structure from per-window slot counts (max over cores)."""
    P = np.zeros(WG + 1, np.int64)
    np.cumsum(Ks, out=P[1:])
    tot = int(P[-1])
    tiles = (tot + 127) // 128
    calls = (tiles + CT - 1) // CT
    b = (P[:-1] // 128).astype(np.int64)          # first tile of window w
    e = ((P[1:] - 1) // 128).astype(np.int64)     # last tile of window w
    ovl = (e - b + 1).astype(np.int64)
    ovl_base = np.zeros(WG + 1, np.int64)
    np.cumsum(ovl, out=ovl_base[1:])
    return P, tot, tiles, calls, b, e, ovl, ovl_base


def _build(Ks: tuple) -> "bacc.Bacc":
    P, TOT, TILES, CALLS, BW, EW, OVL, OVLB = _structure(np.asarray(Ks))
    TOTOVL = int(OVLB[-1])
    OVLMAX = int(OVL.max())
    SLOTCAP = CALLS * CT * 128
    IDXC = (SLOTCAP + 256) // 16

    nc = bacc.Bacc("TRN2", target_bir_lowering=False, debug=False,
                   num_devices=NCORES, dynamic_dma_scratch_size=SCRATCH)

    # ---- inputs ----
    x_sh = nc.dram_tensor("x_sh", [128, WL * CH], dt.float32, kind="ExternalInput")
    dinv_in = nc.dram_tensor("dinv_in", [128, WL], dt.float32, kind="ExternalInput")
    colrel = nc.dram_tensor("colrel", [128, TOTOVL], dt.bfloat16, kind="ExternalInput")
    idx_in = nc.dram_tensor("idx_in", [128, IDXC], dt.int16, kind="ExternalInput")
    mw_in = nc.dram_tensor("mw_in", [64], dt.float32, kind="ExternalInput")
    wih_in = nc.dram_tensor("wih_in", [128, 2 * CH], dt.float32, kind="ExternalInput")
    bih_in = nc.dram_tensor("bih_in", [192], dt.float32, kind="ExternalInput")
    bhh_in = nc.dram_tensor("bhh_in", [192], dt.float32, kind="ExternalInput")
    wtw_in = nc.dram_tensor("wtw_in", [128, 32 * CH], dt.float32, kind="ExternalInput")
    wtb_in = nc.dram_tensor("wtb_in", [4096], dt.float32, kind="ExternalInput")
    gbias_in = nc.dram_tensor("gbias_in", [64], dt.float32, kind="ExternalInput")

    out_d = nc.dram_tensor("out_d", [128, WL * CH], dt.float32, kind="ExternalOutput")

    xd_d = nc.dram_tensor("xd_d", [NPAD_L, CH], dt.float32)
    # per-chunk transposed layout: [segment s, chunk q, partition p, local
    # window lw, ch]; row (q, s*LWS+lw, p) of the padded dst space lives at
    # partial_d[s, q, p, lw, :], so flush writes and the RS-output tail load
    # are contiguous per partition. One ReduceScatter per segment, issued as
    # soon as the segment's windows are flushed, so all but the last RS (and
    # per-segment tail) hide under the continuing aggregation.
    partial_d = nc.dram_tensor("partial_d", [SEG, NCORES, 128, LWS, CH],
                               dt.bfloat16)
    agg_sh = nc.dram_tensor("agg_sh", [SEG, 128, LWS, CH], dt.bfloat16)

    with tile.TileContext(nc) as tc:
        with (
            tc.tile_pool(name="res", bufs=1) as res,
            tc.tile_pool(name="work", bufs=2) as work,
            tc.tile_pool(name="msgsp", bufs=16) as msgsp,
            tc.tile_pool(name="ohp", bufs=2) as ohp,
            tc.tile_pool(name="fbp", bufs=2) as fbp,
        ):
            # ---- head: xd table + idx first, so gathers start early ----
            dinv_sb = res.tile([128, WL], dt.float32)
            nc.sync.dma_start(dinv_sb[:], dinv_in[:])
            xd_sb = res.tile([128, WL, CH], dt.float32)
            XG = 14
            for w0 in range(0, WL, XG):
                w1 = min(w0 + XG, WL)
                xg = work.tile([128, XG, CH], dt.float32, tag="xg")
                nc.sync.dma_start(
                    xg[:, :w1 - w0, :],
                    x_sh[:, w0 * CH:w1 * CH].rearrange("p (w c) -> p w c", c=CH))
                nc.vector.tensor_tensor(
                    out=xd_sb[:, w0:w1, :], in0=xg[:, :w1 - w0, :],
                    in1=dinv_sb[:, w0:w1].unsqueeze(2)
                        .to_broadcast([128, w1 - w0, CH]),
                    op=mybir.AluOpType.mult)
            # table stored p-major (row r at position (r%128)*WL + r//128) so
            # the write is contiguous per partition; idx values are permuted
            # to match on the host.
            nc.sync.dma_start(
                xd_d[:].rearrange("(p w) c -> p w c", w=WL), xd_sb[:])

            idx_sb = res.tile([128, IDXC], dt.int16)
            IDXA = min(24 * CT * 8, IDXC)   # first calls' idx slice
            nc.sync.dma_start(idx_sb[:, :IDXA], idx_in[:, :IDXA])
            nc.sync.dma_start(idx_sb[:, IDXA:], idx_in[:, IDXA:])
            col_sb = res.tile([128, TOTOVL], dt.bfloat16)
            nc.sync.dma_start(col_sb[:], colrel[:])
            iota_b = res.tile([128, 128], dt.bfloat16)
            nc.gpsimd.iota(iota_b[:], pattern=[[1, 128]], base=0,
                           channel_multiplier=0, allow_small_or_imprecise_dtypes=True)
            bias_sb = res.tile([128, CH], dt.float32)
            nc.sync.dma_start(bias_sb[:], gbias_in[None, :].to_broadcast([128, CH]))
            ident = res.tile([128, 128], dt.float32)
            make_identity(nc, ident[:])
            WT_sb = res.tile([64, 64], dt.float32)

            # ---- phase A: W generation (overlaps phase C; W used in tail) ----
            with tc.tile_pool(name="psA", bufs=2, space="PSUM") as psA:
                wih_sb = work.tile([128, 2, CH], dt.float32, tag="wih")
                nc.sync.dma_start(wih_sb[:], wih_in[:].rearrange("p (t c) -> p t c", c=CH))
                wihT_sb = work.tile([64, 256], dt.float32, tag="wihT")
                for t in range(2):
                    trp = psA.tile([64, 128], dt.float32, space="PSUM", tag="tr")
                    nc.tensor.transpose(trp[:], wih_sb[:, t, :], ident[:])
                    nc.vector.tensor_copy(wihT_sb[:, 128 * t:128 * (t + 1)], trp[:])

                mw_sb = work.tile([64, 1], dt.float32, tag="mw")
                nc.sync.dma_start(mw_sb[:], mw_in[:, None])
                bih_sb = work.tile([64, 3], dt.float32, tag="bih")
                nc.sync.dma_start(bih_sb[:], bih_in[:].rearrange("(s p) -> p s", p=64))
                bhh_sb = work.tile([64, 3], dt.float32, tag="bhh")
                nc.sync.dma_start(bhh_sb[:], bhh_in[:].rearrange("(s p) -> p s", p=64))

                gi_sb = work.tile([64, 3], dt.float32, tag="gi")
                for s in range(3):
                    gps = psA.tile([64, 1], dt.float32, space="PSUM", tag="gi")
                    nc.tensor.matmul(gps[:], wihT_sb[:, 64 * s:64 * (s + 1)],
                                     mw_sb[:], start=True, stop=True)
                    nc.vector.tensor_copy(gi_sb[:, s:s + 1], gps[:])

                bsum = work.tile([64, 2], dt.float32, tag="bsum")
                nc.vector.tensor_add(bsum[:], bih_sb[:, 0:2], bhh_sb[:, 0:2])
                gates = work.tile([64, 4], dt.float32, tag="gates")
                nc.scalar.activation(gates[:, 0:1], gi_sb[:, 0:1],
                                     mybir.ActivationFunctionType.Sigmoid,
                                     bias=bsum[:, 0:1])
                nc.scalar.activation(gates[:, 1:2], gi_sb[:, 1:2],
                                     mybir.ActivationFunctionType.Sigmoid,
                                     bias=bsum[:, 1:2])
                nb = work.tile([64, 1], dt.float32, tag="nb")
                nc.vector.tensor_mul(nb[:], gates[:, 0:1], bhh_sb[:, 2:3])
                nc.vector.tensor_add(nb[:], nb[:], bih_sb[:, 2:3])
                nc.scalar.activation(gates[:, 2:3], gi_sb[:, 2:3],
                                     mybir.ActivationFunctionType.Tanh, bias=nb[:])
                omz = work.tile([64, 1], dt.float32, tag="omz")
                nc.vector.tensor_scalar(omz[:], gates[:, 1:2], -1.0, 1.0,
                                        mybir.AluOpType.mult, mybir.AluOpType.add)
                um_sb = work.tile([64, 1], dt.float32, tag="um")
                nc.vector.tensor_mul(um_sb[:], omz[:], gates[:, 2:3])

                wtw_sb = work.tile([128, 32, CH], dt.float32, tag="wtw")
                nc.sync.dma_start(wtw_sb[:], wtw_in[:].rearrange("p (t c) -> p t c", c=CH))
                wtbT_sb = work.tile([64, 64], dt.float32, tag="wtbT")
                nc.sync.dma_start(wtbT_sb[:], wtb_in[:].rearrange("(o p) -> p o", p=64))
                W_ps = psA.tile([64, 64], dt.float32, space="PSUM", tag="W")
                for t in range(32):
                    trp = psA.tile([64, 128], dt.float32, space="PSUM", tag="tr")
                    nc.tensor.transpose(trp[:], wtw_sb[:, t, :], ident[:])
                    trs = work.tile([64, 128], dt.float32, tag="trs")
                    nc.vector.tensor_copy(trs[:], trp[:])
                    for b in range(2):
                        nc.tensor.matmul(W_ps[:, 2 * t + b:2 * t + b + 1],
                                         trs[:, 64 * b:64 * (b + 1)], um_sb[:],
                                         start=True, stop=True,
                                         skip_group_check=True)
                nc.vector.tensor_add(WT_sb[:], W_ps[:], wtbT_sb[:])

            # ---- phase C: gather + aggregate + per-segment RS + tail ----
            def emit_rs(s):
                if not NO_RS:
                    nc.gpsimd.collective_compute(
                        "ReduceScatter", mybir.AluOpType.add,
                        replica_groups=[list(range(NCORES))],
                        ins=[partial_d[s]], outs=[agg_sh[s]])
                else:
                    nc.sync.dma_start(agg_sh[s], partial_d[s, 0])

            def tail_segment(psT, tailp, s):
                agg_sb = tailp.tile([128, LWS, CH], dt.bfloat16, tag="agg")
                nc.sync.dma_start(agg_sb[:], agg_sh[s])
                s_sb = tailp.tile([128, LWS, CH], dt.float32, tag="sseg")
                nc.scalar.copy(s_sb[:], agg_sb[:])
                nc.vector.tensor_add(s_sb[:], s_sb[:],
                                     xd_sb[:, s * LWS:(s + 1) * LWS, :])
                owp = None
                for j in range(LWS):
                    lw = s * LWS + j
                    sTp = psT.tile([64, 128], dt.float32, space="PSUM", tag="sT")
                    nc.tensor.transpose(sTp[:], s_sb[:, j, :], ident[:])
                    sTs = tailp.tile([64, 128], dt.float32, tag="sTs")
                    nc.vector.tensor_copy(sTs[:], sTp[:])
                    if j % FB == 0:
                        owp = psT.tile([128, FB, CH], dt.float32, space="PSUM",
                                       tag="ow")
                    nc.tensor.matmul(owp[:, j % FB, :], sTs[:], WT_sb[:],
                                     start=True, stop=True)
                    if j % FB == FB - 1:
                        j0 = j - (FB - 1)
                        lw0 = lw - (FB - 1)
                        ob = tailp.tile([128, FB, CH], dt.float32, tag="ob")
                        nc.vector.tensor_tensor(
                            out=ob[:], in0=owp[:],
                            in1=dinv_sb[:, lw0:lw + 1].unsqueeze(2)
                                .to_broadcast([128, FB, CH]),
                            op=mybir.AluOpType.mult)
                        nc.vector.tensor_tensor(
                            out=ob[:], in0=ob[:],
                            in1=bias_sb[:].unsqueeze(1)
                                .to_broadcast([128, FB, CH]),
                            op=mybir.AluOpType.add)
                        nc.sync.dma_start(
                            out_d[:, lw0 * CH:(lw + 1) * CH]
                            .rearrange("p (f c) -> p f c", c=CH),
                            ob[:])

            with (
                tc.tile_pool(name="psC", bufs=3, space="PSUM") as psC,
                tc.tile_pool(name="psT", bufs=2, space="PSUM") as psT,
                tc.tile_pool(name="tailp", bufs=3) as tailp,
            ):
                call_tiles = []          # call index -> msg tile object
                nk = (CT + 1) * 128

                def emit_gathers(need_tile):
                    # ensure calls covering global tile index `need_tile`
                    # exist, plus ~10 calls of lookahead so consumers ride out
                    # the 20us the per-segment collective blocks this queue
                    need_tile = min(need_tile + 10 * CT, TILES - 1)
                    while len(call_tiles) * CT <= need_tile:
                        c = len(call_tiles)
                        mt = msgsp.tile([128, CT + 1, CH], dt.float32, tag="msgs")
                        c0 = c * CT * 8  # idx cols per call: CT*128/16
                        nc.gpsimd.dma_gather(
                            mt[:], xd_d[0:, :],
                            idx_sb[:, c0:c0 + nk // 16], nk, nk, CH)
                        call_tiles.append(mt)

                aps = None
                for w in range(WG):
                    s, r = divmod(w, QL)
                    q, lwo = divmod(r, LWS)
                    emit_gathers(int(EW[w]))
                    novl = int(OVL[w])
                    tb = int(OVLB[w])
                    oh = ohp.tile([128, OVLMAX, 128], dt.float32, tag="oh")
                    nc.vector.tensor_tensor(
                        out=oh[:, :novl, :],
                        in0=col_sb[:, tb:tb + novl].unsqueeze(2)
                            .to_broadcast([128, novl, 128]),
                        in1=iota_b[:].unsqueeze(1).to_broadcast([128, novl, 128]),
                        op=mybir.AluOpType.is_equal)
                    if w % FB == 0:
                        aps = psC.tile([128, FB, CH], dt.float32, space="PSUM",
                                       tag="agg")
                    for i in range(novl):
                        j = int(BW[w]) + i
                        mt = call_tiles[j // CT]
                        nc.tensor.matmul(aps[:, w % FB, :], oh[:, i, :],
                                         mt[:, j % CT, :],
                                         start=(i == 0), stop=(i == novl - 1))
                    if w % FB == FB - 1:
                        fb = fbp.tile([128, FB, CH], dt.bfloat16, tag="fb")
                        nc.scalar.copy(fb[:], aps[:])
                        nc.sync.dma_start(
                            partial_d[s, q, :, lwo - (FB - 1):lwo + 1, :],
                            fb[:])
                    # software-pipelined: segment s-1's RS issues a few
                    # windows into segment s (flushes already landed, so no
                    # engine-queue stall) and its tail ~50 windows in (RS
                    # already completed).
                    if r == 2 and s > 0:
                        emit_rs(s - 1)
                    if r == 56 and s > 0:
                        tail_segment(psT, tailp, s - 1)
                emit_rs(SEG - 1)
                tail_segment(psT, tailp, SEG - 1)

    nc.compile()
    return nc


def _host_prep(x, edge_index, memory_weights, gru_w_ih, gru_b_ih, gru_b_hh,
               wt_w, wt_b, gcn_bias):
    rows = np.asarray(edge_index[0], dtype=np.int64)
    cols = np.asarray(edge_index[1], dtype=np.int64)
    x = np.asarray(x, dtype=np.float32)

    deg = np.bincount(cols, minlength=N_NODES).astype(np.float32)
    dinv = 1.0 / np.sqrt(deg + 1.0)

    core = rows // NLOC
    per_core = []
    cnts = np.zeros((NCORES, WG), np.int64)
    for k in range(NCORES):
        sel = core == k
        ec = cols[sel]
        er = rows[sel] - k * NLOC
        # padded dst space: chunk q = col//12500, local i = col%12500,
        # local window lwg = i>>7, in-window dst = i&127. Processing position
        # interleaves segments of LWS local windows across chunks:
        # w = (lwg//LWS)*QL + q*LWS + lwg%LWS
        eq, ei = np.divmod(ec, NLOC)
        lwg = ei >> 7
        w = (lwg // LWS) * QL + eq * LWS + (lwg % LWS)
        order = np.argsort(w, kind="stable")
        ei = ei[order]
        er = er[order]
        w = w[order]
        cnts[k] = np.bincount(w, minlength=WG)
        per_core.append((ei, er, w))
    Ks = np.maximum(cnts.max(axis=0), 1)
    P, TOT, TILES, CALLS, BW, EW, OVL, OVLB = _structure(Ks)
    TOTOVL = int(OVLB[-1])
    SLOTCAP = CALLS * CT * 128
    IDXC = (SLOTCAP + 256) // 16

    in_maps = []
    for k in range(NCORES):
        ei, er, w = per_core[k]
        # rank within window (ec sorted -> consecutive runs per window)
        wstart = np.zeros(WG + 1, np.int64)
        np.cumsum(cnts[k], out=wstart[1:])
        ranks = np.arange(len(ei)) - wstart[w]
        slot = P[w] + ranks

        idxs = np.zeros(SLOTCAP + 256, np.int16)
        idxs[slot] = ((er % 128) * WL + er // 128).astype(np.int16)
        idx_cols = idxs[:IDXC * 16].reshape(IDXC, 16).T
        idx_rep = np.tile(idx_cols, (8, 1)).copy()

        # colrel: per (window, overlap-tile) column of 128 token->dst values
        colrel_arr = np.full((TOTOVL, 128), -1.0, np.float32)
        ocol = OVLB[w] + (slot // 128 - BW[w])
        colrel_arr[ocol, slot % 128] = (ei & 127).astype(np.float32)

        xp = np.zeros((NPAD_L, CH), np.float32)
        xp[:NLOC] = x[k * NLOC:(k + 1) * NLOC]
        x_shuf = xp.reshape(WL, 128, CH).transpose(1, 0, 2).reshape(128, WL * CH).copy()

        dp = np.ones(NPAD_L, np.float32)
        dp[:NLOC] = dinv[k * NLOC:(k + 1) * NLOC]
        dinv_shuf = dp.reshape(WL, 128).T.copy()

        wih_p = np.zeros((256, CH), np.float32)
        wih_p[:192] = np.asarray(gru_w_ih, np.float32)
        wih_shuf = wih_p.reshape(2, 128, CH).transpose(1, 0, 2).reshape(128, 2 * CH).copy()
        wtw = np.asarray(wt_w, np.float32)
        wtw_shuf = wtw.reshape(32, 128, CH).transpose(1, 0, 2).reshape(128, 32 * CH).copy()

        in_maps.append(dict(
            x_sh=x_shuf,
            dinv_in=dinv_shuf,
            colrel=colrel_arr.T.astype(ml_dtypes.bfloat16).copy(),
            idx_in=idx_rep,
            mw_in=np.asarray(memory_weights, np.float32),
            wih_in=wih_shuf,
            bih_in=np.asarray(gru_b_ih, np.float32),
            bhh_in=np.asarray(gru_b_hh, np.float32),
            wtw_in=wtw_shuf,
            wtb_in=np.asarray(wt_b, np.float32),
            gbias_in=np.asarray(gcn_bias, np.float32),
        ))
    return tuple(int(v) for v in Ks), in_maps


def kernel(x, edge_index, memory_weights, gru_w_ih, gru_w_hh, gru_b_ih,
           gru_b_hh, wt_w, wt_b, gcn_bias, _want_trace=False):
    Ks, in_maps = _host_prep(x, edge_index, memory_weights, gru_w_ih,
                             gru_b_ih, gru_b_hh, wt_w, wt_b, gcn_bias)
    if Ks not in _BUILD_CACHE:
        _BUILD_CACHE[Ks] = _build(Ks)
    nc = _BUILD_CACHE[Ks]
    res = run_bass_kernel_spmd(nc, in_maps, list(range(NCORES)),
                               trace=_want_trace)
    out = np.empty((N_NODES, CH), np.float32)
    for j in range(NCORES):
        o = res.results[j]["out_d"].reshape(128, WL, CH).transpose(1, 0, 2)
        out[j * NLOC:(j + 1) * NLOC] = o.reshape(NPAD_L, CH)[:NLOC]
    kernel._last_result = res
    return out
